# revision 1
# baseline (speedup 1.0000x reference)
"""Batched multi-head graph attention (GAT) kernel for 8 Trainium2 NeuronCores.

Math (per batch b, head h):
    hp      = h[b] @ w[h]                          # [N, F]
    t       = tanh(hp)
    s       = t @ a_src[h];  d = t @ a_dst[h]      # [N]
    score   = leaky_relu(s_i + d_j, 0.2)
    e       = where(adj>0, exp(score), 0)
    out     = (e / e.sum(-1, keepdim)) @ hp + bias

Key identities used on-device:
    exp(leaky(z)) = max(exp(z), exp(0.2 z))                      (slope < 1)
                  = e^{0.2 s_i} * max(e^{0.8 s_i} e^{d_j}, e^{0.2 d_j})
    The e^{0.2 s_i} factor is constant along j, so it cancels in the
    softmax ratio.  With q=e^{0.8s}, v=e^d, v2=e^{0.2d} the masked weight is
        D[j,i] = adj[i,j] * max(q_i v_j, v2_j)     (up to a row-constant)
    computed per 128x1024 tile as ONE dual-scalar op + ONE mask multiply
    (fp16).  A single PE matmul against lhsT=[hp | 1] accumulates numerator
    and denominator together into PSUM [65, 512].

adj mask trick: adj values are exactly 0.0/1.0 fp32 = 0x00000000/0x3F800000.
The low 16 bits are zero, and the high 16 bits (0x3F80) read as fp16 equal
1.875 -- a constant scale on every surviving softmax term, which cancels in
the normalization.  So the host passes the high uint16 halves (half the
bytes) and the device DMA-transposes them directly as the fp16 mask.

Sharding: 8 cores = 4 batches x 2 query-row halves; each core handles all 4
heads for its 1024 query rows against all 2048 keys.  Keys are rotated on
host so each core's queries are local rows [0, 1024).
"""

import os
from contextlib import ExitStack

import numpy as np

import concourse.bass as bass
import concourse.mybir as mybir
import concourse.tile as tile
from concourse import bacc
from concourse.bass_utils import run_bass_kernel_spmd
from concourse.masks import make_identity

F32 = mybir.dt.float32
F16 = mybir.dt.float16
U16 = mybir.dt.uint16
ALU = mybir.AluOpType
ACTF = mybir.ActivationFunctionType
AX = mybir.AxisListType

B, N, H, F = 4, 2048, 4, 64
NCORES = 8
ROWS = N // 2          # query rows per core
KEYS = N               # keys per core (full)
NEG_SLOPE = 0.2


def default_assign(jb, h):
    """E-tile source for head h.

    "dve": E = max(q*v, v2) via one DVE tensor_scalar (full weight).
    "act": E = relu(q*v - v2) via one ACT pass; the missing v2*adjT
           contribution is added by an extra matmul whose weights are
           the v2-scaled [hp | 1] (exact, since adjT >= 0).
    """
    if h % 2 == 1:
        return "act"
    return "act" if jb in (3, 5, 7, 11, 14) else "dve"


def build_program(rows=ROWS, keys=KEYS, heads=H, f=F, assign=default_assign,
                  sd_engine="vector"):
    nc = bacc.Bacc("TRN2", target_bir_lowering=False, debug=False)

    kb = keys // 128          # key blocks
    qb = rows // 128          # query blocks
    nhalf = rows // 512       # output column halves (psum tiles per head)
    fe = f + 1                # hp with ones column appended

    hb_d = nc.dram_tensor("hb", [keys, f], F32, kind="ExternalInput")
    adjh_d = nc.dram_tensor("adjh", [rows, keys], U16, kind="ExternalInput")
    w_d = nc.dram_tensor("wmat", [heads, f, f], F32, kind="ExternalInput")
    ap_d = nc.dram_tensor("apairt", [heads, 2, f], F32, kind="ExternalInput")
    out_d = nc.dram_tensor("out", [heads, rows, f], F32,
                           kind="ExternalOutput")

    eng = {"dve": nc.vector, "gps": nc.gpsimd}
    sd_eng = nc.vector if sd_engine == "vector" else nc.gpsimd

    with tile.TileContext(nc) as tc:
        with (
            tc.tile_pool(name="const", bufs=1) as const,
            tc.tile_pool(name="persist", bufs=1) as persist,
            tc.tile_pool(name="stmp", bufs=3) as stmp,
        ):
            id16 = const.tile([128, 128], F16, tag="id16")
            make_identity(nc, id16)
            id32 = const.tile([128, 128], F32, tag="id32")
            make_identity(nc, id32)

            # ---- global loads -------------------------------------------
            h32 = persist.tile([128, kb, f], F32, tag="h32")
            nc.sync.dma_start(
                out=h32, in_=hb_d.ap().rearrange("(t p) f -> p t f", p=128))
            h16 = persist.tile([128, kb, f], F16, tag="h16")
            nc.vector.tensor_copy(h16, h32)

            w32 = persist.tile([f, heads, f], F32, tag="w32")
            nc.sync.dma_start(out=w32, in_=w_d.ap().rearrange("h f o -> f h o"))
            w16 = persist.tile([f, heads, f], F16, tag="w16")
            nc.vector.tensor_copy(w16, w32)

            apr32 = persist.tile([1, heads, 2, f], F32, tag="apr32")
            nc.sync.dma_start(out=apr32, in_=ap_d.ap().unsqueeze(0))
            abc32 = persist.tile([128, heads, 2, f], F32, tag="abc32")
            nc.gpsimd.partition_broadcast(abc32, apr32)
            a16 = persist.tile([128, heads, 2, f], F16, tag="a16")
            nc.vector.tensor_copy(a16, abc32)

            # ---- hT (transposed h, fp16) --------------------------------
            hT16 = persist.tile([64, keys], F16, tag="hT16")
            g_ht = min(4, kb)
            with tc.tile_pool(name="psum_ht", bufs=2, space="PSUM") as pht:
                for g in range(kb // g_ht):
                    pt = pht.tile([64, g_ht * 128], F16, tag="pht")
                    for t in range(g_ht):
                        blk = g * g_ht + t
                        nc.tensor.transpose(
                            pt[:, t * 128:(t + 1) * 128],
                            h16[:, blk, :], id16)
                    nc.vector.tensor_copy(
                        hT16[:, g * g_ht * 128:(g + 1) * g_ht * 128], pt)

            # ---- per-head setup -----------------------------------------
            hpt = []   # [128, kb, fe] fp16 -- [hp | 1] in key-block layout
            qbc = []   # [128, rows] fp16 -- exp(0.8 s_i) broadcast
            vv, vv2, nvv2 = [], [], []
            hpt2 = {}  # v2-scaled [hp | 1] for "act"-path heads
            with (
                tc.tile_pool(name="psum_hp", bufs=2, space="PSUM") as php,
                tc.tile_pool(name="psum_q", bufs=2, space="PSUM") as pq,
            ):
                for h in range(heads):
                    hpt_h = persist.tile([128, kb, fe], F16, tag=f"hpt{h}")
                    tanh_h = stmp.tile([128, kb, f], F16, tag="tanh")
                    g_hp = min(8, kb)
                    for k in range(kb // g_hp):
                        pp = php.tile([128, g_hp * f], F32, tag="php")
                        for t in range(g_hp):
                            blk = k * g_hp + t
                            nc.tensor.matmul(
                                pp[:, t * f:(t + 1) * f],
                                lhsT=hT16[:, blk * 128:(blk + 1) * 128],
                                rhs=w16[:, h, :], start=True, stop=True)
                        nc.scalar.activation(
                            hpt_h[:, k * g_hp:(k + 1) * g_hp, 0:f],
                            pp.rearrange("p (t o) -> p t o", o=f),
                            ACTF.Identity)
                        nc.scalar.activation(
                            tanh_h[:, k * g_hp:(k + 1) * g_hp, :],
                            pp.rearrange("p (t o) -> p t o", o=f),
                            ACTF.Tanh)
                    nc.vector.memset(hpt_h[:, :, f:fe], 1.0)
                    hpt.append(hpt_h)

                    # s, d via elementwise mul + per-block reduce
                    prod = stmp.tile([128, kb, 2, f], F16, tag="prod")
                    sd_eng.tensor_tensor(
                        out=prod,
                        in0=tanh_h.unsqueeze(2).broadcast_to([128, kb, 2, f]),
                        in1=a16[:, h].unsqueeze(1).broadcast_to(
                            [128, kb, 2, f]),
                        op=ALU.mult)
                    # split the reduction: the s-part (first qb blocks) is
                    # all the q-chain needs, so it unblocks q/broadcast
                    # before the full d reduction finishes
                    sums = stmp.tile([128, kb, 2], F32, tag="sums")
                    sd_eng.reduce_sum(sums[:, 0:qb, 0:1],
                                      prod[:, 0:qb, 0:1, :], axis=AX.X)
                    sd_eng.reduce_sum(sums[:, :, 1:2],
                                      prod[:, :, 1:2, :], axis=AX.X)

                    v_h = persist.tile([128, kb], F32, tag=f"v{h}")
                    v2_h = persist.tile([128, kb], F32, tag=f"v2{h}")
                    nv2_h = persist.tile([128, kb], F32, tag=f"nv2{h}")
                    nc.scalar.activation(v_h, sums[:, :, 1], ACTF.Exp)
                    nc.scalar.activation(v2_h, sums[:, :, 1], ACTF.Exp,
                                         scale=NEG_SLOPE)
                    nc.vector.tensor_scalar_mul(nv2_h, v2_h, -1.0)
                    vv.append(v_h)
                    vv2.append(v2_h)
                    nvv2.append(nv2_h)

                    if any(assign(j, h) == "act" for j in range(kb)):
                        # v2-scaled [hp | 1]: weights for the matmul that
                        # restores the v2*adjT part of max(qv, v2)*adjT
                        hpt2_h = persist.tile([128, kb, fe], F16,
                                              tag=f"hpt2{h}")
                        nc.vector.tensor_tensor(
                            out=hpt2_h, in0=hpt_h,
                            in1=v2_h.unsqueeze(2).broadcast_to(
                                [128, kb, fe]),
                            op=ALU.mult)
                        hpt2[h] = hpt2_h

                    # q = exp(0.8 s) over this core's query rows, broadcast
                    pq_t = pq.tile([1, rows], F32, tag="pq")
                    for t in range(qb):
                        nc.tensor.transpose(
                            pq_t[:, t * 128:(t + 1) * 128],
                            sums[:, t:t + 1, 0:1], id32)
                    qrow = stmp.tile([1, rows], F16, tag="qrow")
                    nc.scalar.activation(qrow, pq_t, ACTF.Exp,
                                         scale=1.0 - NEG_SLOPE)
                    qb_h = persist.tile([128, rows], F16, tag=f"qb{h}")
                    nc.gpsimd.partition_broadcast(qb_h, qrow)
                    qbc.append(qb_h)

            # ---- main loop: masked weights + fused matmul ----------------
            # heads run in PAIRS so pair 2's setup overlaps pair 1's loop
            # and pair 1's normalize/store overlaps pair 2's loop.
            nacc = heads * nhalf
            acc_sb = persist.tile([fe, nacc, 512], F32, tag="acc_sb")
            pairs = [tuple(range(p, min(p + 2, heads)))
                     for p in range(0, heads, 2)]
            nq = 512 // 128  # transpose chunks per acc tile

            with (
                tc.tile_pool(name="adjp", bufs=kb) as adjp,
                tc.tile_pool(name="ep", bufs=4) as ep,
                tc.tile_pool(name="dp", bufs=4) as dp,
                tc.tile_pool(name="outp", bufs=4) as outp,
                ExitStack() as pools,
            ):
                # prefetch ALL transposed mask blocks up front (resident)
                adjts = []
                for jb in range(kb):
                    adjt = adjp.tile([128, rows], U16, tag="adjt",
                                     name=f"adjt{jb}")
                    nc.sync.dma_start_transpose(
                        adjt, adjh_d.ap()[:, jb * 128:(jb + 1) * 128])
                    adjts.append(adjt)

                # PSUM pools are stack-allocated: open acc pools in reverse
                # pair order so pair 0's closes first (LIFO), letting its
                # normalize PSUM reuse those banks while pair 1 still
                # accumulates in its own.
                acc_stacks = {}
                accps = {}
                for pi in reversed(range(len(pairs))):
                    st = ExitStack()
                    acc_stacks[pi] = st
                    accps[pi] = st.enter_context(
                        tc.tile_pool(name=f"accp{pi}", bufs=1, space="PSUM"))
                accs = {}
                for pi, pair in enumerate(pairs):
                    for h in pair:
                        for half in range(nhalf):
                            i = h * nhalf + half
                            accs[i] = accps[pi].tile(
                                [fe, 512], F32, tag=f"acc{i}",
                                name=f"acc{i}")

                for pi, pair in enumerate(pairs):
                    np_ = len(pair)
                    for jb in range(kb):
                        adj16 = adjts[jb].bitcast(F16)
                        ea = ep.tile([128, np_, rows], F16, tag="ea")
                        da = dp.tile([128, np_, rows], F16, tag="da")
                        for k, h in enumerate(pair):
                            v_s = vv[h][:, jb:jb + 1]
                            v2_s = vv2[h][:, jb:jb + 1]
                            if assign(jb, h) == "act":
                                nc.scalar.activation(
                                    ea[:, k, :], qbc[h], ACTF.Relu,
                                    bias=nvv2[h][:, jb:jb + 1], scale=v_s)
                            else:
                                nc.vector.tensor_scalar(
                                    out=ea[:, k, :], in0=qbc[h],
                                    scalar1=v_s, scalar2=v2_s,
                                    op0=ALU.mult, op1=ALU.max)
                        nc.vector.tensor_tensor(
                            out=da, in0=ea,
                            in1=adj16.unsqueeze(1).broadcast_to(
                                [128, np_, rows]),
                            op=ALU.mult)
                        dsrc = da
                        for k, h in enumerate(pair):
                            is_act = assign(jb, h) == "act"
                            for half in range(nhalf):
                                rhs_slice = slice(half * 512,
                                                  (half + 1) * 512)
                                nc.tensor.matmul(
                                    accs[h * nhalf + half],
                                    lhsT=hpt[h][:, jb, :],
                                    rhs=dsrc[:, k, rhs_slice],
                                    start=(jb == 0),
                                    stop=(jb == kb - 1 and not is_act))
                                if is_act:
                                    nc.tensor.matmul(
                                        accs[h * nhalf + half],
                                        lhsT=hpt2[h][:, jb, :],
                                        rhs=adj16[:, rhs_slice],
                                        start=False, stop=(jb == kb - 1))

                    # spill this pair's accumulators to SBUF (PE reads SBUF
                    # only); alternate engines so copies drain in parallel
                    for h in pair:
                        for half in range(nhalf):
                            i = h * nhalf + half
                            nc.scalar.activation(
                                acc_sb[:, i, :], accs[i], ACTF.Identity)
                    # free this pair's PSUM banks, then normalize this pair
                    # in transposed [i, o] form (overlaps next pair's loop)
                    acc_stacks[pi].close()
                    ptf_st = ExitStack()
                    ptf = ptf_st.enter_context(
                        tc.tile_pool(name=f"ptf{pi}", bufs=2, space="PSUM"))
                    for h in pair:
                        for half in range(nhalf):
                            i = h * nhalf + half
                            pt = ptf.tile([128, nq, fe], F32, tag=f"pt{pi}")
                            for q in range(nq):
                                nc.tensor.transpose(
                                    pt[:, q, :],
                                    acc_sb[:, i, q * 128:(q + 1) * 128],
                                    id32[0:fe, 0:fe])
                            rcol = outp.tile([128, nq], F32, tag="rcol")
                            nc.vector.reciprocal(rcol, pt[:, :, f])
                            osb = outp.tile([128, nq, f], F32, tag="osb")
                            nc.vector.tensor_tensor(
                                out=osb, in0=pt[:, :, 0:f],
                                in1=rcol.unsqueeze(2).broadcast_to(
                                    [128, nq, f]),
                                op=ALU.mult)
                            nc.sync.dma_start(
                                out=out_d.ap()[
                                    h, half * 512:(half + 1) * 512, :]
                                .rearrange("(q p) f -> p q f", p=128),
                                in_=osb)
                    ptf_st.close()
    nc.compile()
    return nc


_PROGRAM_CACHE = {}


def _get_program():
    key = "full"
    if key not in _PROGRAM_CACHE:
        _PROGRAM_CACHE[key] = build_program()
    return _PROGRAM_CACHE[key]


def make_in_maps(h, adj, w, a_src, a_dst):
    """Shard + marshal the full inputs into 8 per-core input maps."""
    h = np.ascontiguousarray(np.asarray(h, dtype=np.float32))
    adj = np.ascontiguousarray(np.asarray(adj, dtype=np.float32))
    w = np.ascontiguousarray(np.asarray(w, dtype=np.float32))
    apairt = np.ascontiguousarray(
        np.concatenate([np.asarray(a_src)[:, None, :, 0],
                        np.asarray(a_dst)[:, None, :, 0]],
                       axis=1).astype(np.float32))  # [H, 2, F]
    in_maps = []
    for c in range(NCORES):
        b, r0 = c // 2, (c % 2) * ROWS
        hb = np.concatenate([h[b, r0:], h[b, :r0]], axis=0)  # rotate keys
        adj_rows = adj[b, r0:r0 + ROWS]
        adj_rot = np.concatenate([adj_rows[:, r0:], adj_rows[:, :r0]], axis=1)
        adjh = np.ascontiguousarray(
            adj_rot.view(np.uint16).reshape(ROWS, KEYS, 2)[:, :, 1])
        in_maps.append({
            "hb": np.ascontiguousarray(hb),
            "adjh": adjh,
            "wmat": w,
            "apairt": apairt,
        })
    return in_maps


def assemble_output(results, bias):
    """Gather per-core [H, ROWS, F] results into [B, H, N, F]."""
    out = np.empty((B, H, N, F), dtype=np.float32)
    for c in range(NCORES):
        b, r0 = c // 2, (c % 2) * ROWS
        out[b, :, r0:r0 + ROWS, :] = results[c]["out"]
    if bias is not None:
        out = out + np.asarray(bias, dtype=np.float32)[None, None, None, :]
    return out


def run(h, adj, w, a_src, a_dst, bias, trace=False, trace_kwargs=None):
    nc = _get_program()
    in_maps = make_in_maps(h, adj, w, a_src, a_dst)
    res = run_bass_kernel_spmd(nc, in_maps, core_ids=list(range(NCORES)),
                               trace=trace, **(trace_kwargs or {}))
    return assemble_output(res.results, bias), res


def kernel(h, adj, w, a_src, a_dst, bias):
    out, _ = run(h, adj, w, a_src, a_dst, bias,
                 trace=bool(int(os.environ.get("GAT_TRACE", "0"))))
    return out



# revision 5
# speedup vs baseline: 1.0292x; 1.0292x over previous
"""Batched multi-head graph attention (GAT) kernel for 8 Trainium2 NeuronCores.

Math (per batch b, head h):
    hp      = h[b] @ w[h]                          # [N, F]
    t       = tanh(hp)
    s       = t @ a_src[h];  d = t @ a_dst[h]      # [N]
    score   = leaky_relu(s_i + d_j, 0.2)
    e       = where(adj>0, exp(score), 0)
    out     = (e / e.sum(-1, keepdim)) @ hp + bias

On-device identity (v2-folded form):
    exp(leaky(z)) = e^{0.2 s_i} * v2_j * max(q_i w_j, 1)
    with q = e^{0.8 s}, w = e^{0.8 d}, v2 = e^{0.2 d}.  The e^{0.2 s_i}
    row factor cancels in softmax.  The v2_j column factor is folded into
    the PE stationary [v2*hp | v2], so the per-element work is ONE
    tensor_scalar (4x DVE mode):  ea = max(q_i * w_j, 1)
    then ONE mask multiply (2x DVE):  da = ea * adjT
    and a PE matmul accumulating numerator and denominator together.

    ACT-offload variant for some (h,jb) blocks: ea_act = relu(q w - 1) on
    the Scalar engine; the missing "+1" is restored by an extra matmul of
    the SAME stationary [v2*hp | v2] against the raw adjT mask (exact).

adj mask trick: adj values 0.0/1.0 fp32; the high uint16 halves bitcast to
fp16 {0, 1.875} -- a constant scale on every surviving term that cancels
in the normalization.  Host passes adj TRANSPOSED as uint16 high halves.

Sharding: 8 cores = 4 batches x 2 head-pairs; each core handles 2 heads for
ALL 2048 query rows against all 2048 keys.  Output is fp16 (host upcasts);
the PSUM spill is scaled by 2^-6 which cancels in the num/den ratio.
"""

import os
from contextlib import ExitStack

import numpy as np

import concourse.bass as bass
import concourse.mybir as mybir
import concourse.tile as tile
from concourse import bacc
from concourse.bass_utils import run_bass_kernel_spmd
from concourse.masks import make_identity

F32 = mybir.dt.float32
F16 = mybir.dt.float16
U16 = mybir.dt.uint16
ALU = mybir.AluOpType
ACTF = mybir.ActivationFunctionType
AX = mybir.AxisListType

B, N, H, F = 4, 2048, 4, 64
NCORES = 8
ROWS = N               # query rows per core (full)
KEYS = N               # keys per core (full)
HEADS_PER = 2          # heads per core
NEG_SLOPE = 0.2
SPILL_SCALE = 2.0 ** -6


def default_assign(jb, h):
    """ea engine for block (jb, head h): "act" -> Scalar relu + PE fixup,
    "dve" -> Vector tensor_scalar max."""
    return "act" if (jb + h) % 2 == 0 else "dve"


def build_program(rows=ROWS, keys=KEYS, heads=HEADS_PER, f=F,
                  assign=default_assign):
    nc = bacc.Bacc("TRN2", target_bir_lowering=False, debug=False)

    kb = keys // 128           # key blocks
    qb = rows // 128           # query blocks (for q transposes)
    nhalf = rows // 512        # 512-wide output column chunks per head
    fe = f + 1                 # [v2*hp | v2] stationary width

    hbT_d = nc.dram_tensor("hbT", [f, keys], F32, kind="ExternalInput")
    adjt_d = nc.dram_tensor("adjt", [keys, rows], U16, kind="ExternalInput")
    w_d = nc.dram_tensor("wmat", [heads, f, f], F32, kind="ExternalInput")
    ap_d = nc.dram_tensor("apairt", [heads, 2, f], F32, kind="ExternalInput")
    out_d = nc.dram_tensor("out", [heads, rows, f], F16,
                           kind="ExternalOutput")

    with tile.TileContext(nc) as tc:
        with (
            tc.tile_pool(name="const", bufs=1) as const,
            tc.tile_pool(name="persist", bufs=1) as persist,
            tc.tile_pool(name="stmp", bufs=2) as stmp,
        ):
            id16 = const.tile([128, 128], F16, tag="id16")
            make_identity(nc, id16)
            neg1 = const.tile([128, 1], F32, tag="neg1")
            nc.vector.memset(neg1, -1.0)

            # ---- global loads -------------------------------------------
            hT32 = persist.tile([f, keys], F32, tag="hT32")
            nc.sync.dma_start(out=hT32, in_=hbT_d.ap())
            hT16 = persist.tile([f, keys], F16, tag="hT16")
            nc.scalar.activation(hT16, hT32, ACTF.Copy)

            w32 = persist.tile([f, heads, f], F32, tag="w32")
            nc.sync.dma_start(out=w32, in_=w_d.ap().rearrange("h f o -> f h o"))
            w16 = persist.tile([f, heads, f], F16, tag="w16")
            nc.vector.tensor_copy(w16, w32)

            apr32 = persist.tile([1, heads, 2, f], F32, tag="apr32")
            nc.sync.dma_start(out=apr32, in_=ap_d.ap().unsqueeze(0))
            abc32 = persist.tile([128, heads, 2, f], F32, tag="abc32")
            nc.gpsimd.partition_broadcast(abc32, apr32)
            a16 = persist.tile([128, heads, 2, f], F16, tag="a16")
            nc.vector.tensor_copy(a16, abc32)

            # ---- prefetch all transposed mask blocks (streamed pool) ----
            # (declared early so DMA runs under the whole setup phase)
            adjp_stack = ExitStack()
            adjp = adjp_stack.enter_context(
                tc.tile_pool(name="adjp", bufs=6))
            adjts = []
            for jb in range(kb):
                adjt_t = adjp.tile([128, rows], U16, tag="adjt",
                                   name=f"adjt{jb}")
                nc.sync.dma_start(
                    out=adjt_t, in_=adjt_d.ap()[jb * 128:(jb + 1) * 128, :])
                adjts.append(adjt_t)

            # ---- per-head setup -----------------------------------------
            tanh16 = persist.tile([128, heads, kb, f], F16, tag="tanh16")
            hpt2 = []    # [128, kb, fe] f16 -- [v2*hp | v2] stationaries
            wcol = []    # [128, kb] f32 -- e^{0.8 d}
            qbc = []     # [128, rows] f16 -- e^{0.8 s} broadcast
            hptmp = []
            with tc.tile_pool(name="psum_hp", bufs=2, space="PSUM") as php:
                for h in range(heads):
                    hpt_h = stmp.tile([128, kb, fe], F16, name=f"hptmp{h}",
                                      tag=f"hptmp{h}")
                    g_hp = min(8, kb)
                    for g in range(kb // g_hp):
                        pp = php.tile([128, g_hp * f], F32, tag="php")
                        for t in range(g_hp):
                            blk = g * g_hp + t
                            nc.tensor.matmul(
                                pp[:, t * f:(t + 1) * f],
                                lhsT=hT16[:, blk * 128:(blk + 1) * 128],
                                rhs=w16[:, h, :], start=True, stop=True)
                        nc.scalar.activation(
                            hpt_h[:, g * g_hp:(g + 1) * g_hp, 0:f],
                            pp.rearrange("p (t o) -> p t o", o=f),
                            ACTF.Identity)
                        nc.scalar.activation(
                            tanh16[:, h, g * g_hp:(g + 1) * g_hp, :],
                            pp.rearrange("p (t o) -> p t o", o=f),
                            ACTF.Tanh)
                    nc.vector.memset(hpt_h[:, :, f:fe], 1.0)
                    hptmp.append(hpt_h)

            # s, d for both heads: prod + blockwise reduce
            prod = stmp.tile([128, heads, kb, 2, f], F16, tag="prod")
            nc.vector.tensor_tensor(
                out=prod,
                in0=tanh16.unsqueeze(3).broadcast_to([128, heads, kb, 2, f]),
                in1=a16.unsqueeze(2).broadcast_to([128, heads, kb, 2, f]),
                op=ALU.mult)
            sums = stmp.tile([128, heads, kb, 2], F32, tag="sums")
            # s first: unblocks the q chain before d finishes
            nc.vector.reduce_sum(sums[:, :, :, 0:1], prod[:, :, :, 0:1, :],
                                 axis=AX.X)
            nc.vector.reduce_sum(sums[:, :, :, 1:2], prod[:, :, :, 1:2, :],
                                 axis=AX.X)

            with tc.tile_pool(name="psum_q", bufs=2, space="PSUM") as pq:
                for h in range(heads):
                    # q chain: exp col (f16), transpose to row, broadcast
                    qc16 = stmp.tile([128, kb], F16, name=f"qc{h}",
                                     tag=f"qc{h}")
                    nc.scalar.activation(qc16, sums[:, h, :, 0], ACTF.Exp,
                                         scale=1.0 - NEG_SLOPE)
                    pq_t = pq.tile([1, rows], F16, tag="pq")
                    for t in range(qb):
                        nc.tensor.transpose(
                            pq_t[:, t * 128:(t + 1) * 128],
                            qc16[:, t:t + 1], id16)
                    qrow = stmp.tile([1, rows], F16, name=f"qrow{h}",
                                     tag=f"qrow{h}")
                    nc.vector.tensor_copy(qrow, pq_t)
                    qb_h = persist.tile([128, rows], F16, tag=f"qb{h}")
                    nc.gpsimd.partition_broadcast(qb_h, qrow)
                    qbc.append(qb_h)

                    # w = e^{0.8 d} (scalar for ea), v2 = e^{0.2 d} (f16,
                    # folded into the stationary)
                    w_h = persist.tile([128, kb], F32, tag=f"w{h}")
                    nc.scalar.activation(w_h, sums[:, h, :, 1], ACTF.Exp,
                                         scale=1.0 - NEG_SLOPE)
                    wcol.append(w_h)
                    v2c = stmp.tile([128, kb], F16, name=f"v2c{h}",
                                    tag=f"v2c{h}")
                    nc.scalar.activation(v2c, sums[:, h, :, 1], ACTF.Exp,
                                         scale=NEG_SLOPE)
                    hpt2_h = persist.tile([128, kb, fe], F16, tag=f"hpt2{h}")
                    nc.vector.tensor_tensor(
                        out=hpt2_h, in0=hptmp[h],
                        in1=v2c.unsqueeze(2).broadcast_to([128, kb, fe]),
                        op=ALU.mult)
                    hpt2.append(hpt2_h)

            # ---- main loop: masked weights + fused matmul ---------------
            nacc = heads * nhalf
            acc_sb = persist.tile([fe, nacc, 512], F16, tag="acc_sb")
            accp_stack = ExitStack()
            accp = accp_stack.enter_context(
                tc.tile_pool(name="accp", bufs=1, space="PSUM"))
            accs = {}
            for h in range(heads):
                for half in range(nhalf):
                    i = h * nhalf + half
                    accs[i] = accp.tile([fe, 512], F32, tag=f"acc{i}",
                                        name=f"acc{i}")

            with (
                tc.tile_pool(name="ep", bufs=3) as ep,
                tc.tile_pool(name="dp", bufs=3) as dp,
            ):
                for jb in range(kb):
                    adj16 = adjts[jb].bitcast(F16)
                    ea = ep.tile([128, heads, rows], F16, tag="ea")
                    for h in range(heads):
                        w_s = wcol[h][:, jb:jb + 1]
                        if assign(jb, h) == "act":
                            nc.scalar.activation(
                                ea[:, h, :], qbc[h], ACTF.Relu,
                                bias=neg1, scale=w_s)
                        else:
                            nc.vector.tensor_scalar(
                                out=ea[:, h, :], in0=qbc[h],
                                scalar1=w_s, scalar2=1.0,
                                op0=ALU.mult, op1=ALU.max)
                    da = dp.tile([128, heads, rows], F16, tag="da")
                    nc.vector.tensor_tensor(
                        out=da, in0=ea,
                        in1=adj16.unsqueeze(1).broadcast_to(
                            [128, heads, rows]),
                        op=ALU.mult)
                    for h in range(heads):
                        is_act = assign(jb, h) == "act"
                        last = jb == kb - 1
                        for half in range(nhalf):
                            sl = slice(half * 512, (half + 1) * 512)
                            nc.tensor.matmul(
                                accs[h * nhalf + half],
                                lhsT=hpt2[h][:, jb, :],
                                rhs=da[:, h, sl],
                                start=(jb == 0),
                                stop=(last and not is_act))
                            if is_act:
                                nc.tensor.matmul(
                                    accs[h * nhalf + half],
                                    lhsT=hpt2[h][:, jb, :],
                                    rhs=adj16[:, sl],
                                    start=False, stop=last)

                # spill accumulators (scaled; scale cancels in num/den)
                for i in range(nacc):
                    nc.scalar.activation(acc_sb[:, i, :], accs[i],
                                         ACTF.Identity, scale=SPILL_SCALE)
            accp_stack.close()
            adjp_stack.close()

            # ---- normalize + store (fp16) -------------------------------
            nq = 512 // 128
            with (
                tc.tile_pool(name="ptf", bufs=2, space="PSUM") as ptf,
                tc.tile_pool(name="outp", bufs=4) as outp,
            ):
                for h in range(heads):
                    for half in range(nhalf):
                        i = h * nhalf + half
                        pt = ptf.tile([128, nq, fe + 1], F16, tag="pt")
                        for q in range(nq):
                            nc.tensor.transpose(
                                pt[:, q, 0:fe],
                                acc_sb[:, i, q * 128:(q + 1) * 128],
                                id16[0:fe, 0:fe])
                        rcol = outp.tile([128, nq], F32, tag="rcol")
                        nc.vector.reciprocal(rcol, pt[:, :, f])
                        rc16 = outp.tile([128, nq], F16, tag="rc16")
                        nc.vector.tensor_copy(rc16, rcol)
                        osb = outp.tile([128, nq, f], F16, tag="osb")
                        nc.vector.tensor_tensor(
                            out=osb, in0=pt[:, :, 0:f],
                            in1=rc16.unsqueeze(2).broadcast_to(
                                [128, nq, f]),
                            op=ALU.mult)
                        nc.sync.dma_start(
                            out=out_d.ap()[
                                h, half * 512:(half + 1) * 512, :]
                            .rearrange("(q p) f -> p q f", p=128),
                            in_=osb)
    nc.compile()
    return nc


_PROGRAM_CACHE = {}


def _get_program():
    key = "full"
    if key not in _PROGRAM_CACHE:
        _PROGRAM_CACHE[key] = build_program()
    return _PROGRAM_CACHE[key]


def make_in_maps(h, adj, w, a_src, a_dst):
    """Shard + marshal the full inputs into 8 per-core input maps."""
    h = np.ascontiguousarray(np.asarray(h, dtype=np.float32))
    adj = np.ascontiguousarray(np.asarray(adj, dtype=np.float32))
    w = np.ascontiguousarray(np.asarray(w, dtype=np.float32))
    apairt_all = np.concatenate(
        [np.asarray(a_src)[:, None, :, 0], np.asarray(a_dst)[:, None, :, 0]],
        axis=1).astype(np.float32)  # [H, 2, F]
    in_maps = []
    hbT = {}
    adjt = {}
    for b in range(B):
        hbT[b] = np.ascontiguousarray(h[b].T)  # [F, N]
        adjT = np.ascontiguousarray(adj[b].T)  # [N, N] keys x queries
        adjt[b] = np.ascontiguousarray(
            adjT.view(np.uint16).reshape(N, N, 2)[:, :, 1])
    for c in range(NCORES):
        b, p = c // 2, c % 2
        in_maps.append({
            "hbT": hbT[b],
            "adjt": adjt[b],
            "wmat": np.ascontiguousarray(w[2 * p:2 * p + 2]),
            "apairt": np.ascontiguousarray(apairt_all[2 * p:2 * p + 2]),
        })
    return in_maps


def assemble_output(results, bias):
    """Gather per-core [2, N, F] fp16 results into [B, H, N, F] fp32."""
    out = np.empty((B, H, N, F), dtype=np.float32)
    for c in range(NCORES):
        b, p = c // 2, c % 2
        out[b, 2 * p:2 * p + 2, :, :] = results[c]["out"].astype(np.float32)
    if bias is not None:
        out = out + np.asarray(bias, dtype=np.float32)[None, None, None, :]
    return out


def run(h, adj, w, a_src, a_dst, bias, trace=False, trace_kwargs=None):
    nc = _get_program()
    in_maps = make_in_maps(h, adj, w, a_src, a_dst)
    res = run_bass_kernel_spmd(nc, in_maps, core_ids=list(range(NCORES)),
                               trace=trace, **(trace_kwargs or {}))
    return assemble_output(res.results, bias), res


def kernel(h, adj, w, a_src, a_dst, bias):
    out, _ = run(h, adj, w, a_src, a_dst, bias,
                 trace=bool(int(os.environ.get("GAT_TRACE", "0"))))
    return out


# revision 10
# speedup vs baseline: 1.0310x; 1.0017x over previous
"""Batched multi-head graph attention (GAT) kernel for 8 Trainium2 NeuronCores.

Math (per batch b, head h):
    hp      = h[b] @ w[h]                          # [N, F]
    t       = tanh(hp)
    s       = t @ a_src[h];  d = t @ a_dst[h]      # [N]
    score   = leaky_relu(s_i + d_j, 0.2)
    e       = where(adj>0, exp(score), 0)
    out     = (e / e.sum(-1, keepdim)) @ hp + bias

On-device identity (v2-folded form):
    exp(leaky(z)) = e^{0.2 s_i} * v2_j * max(q_i w_j, 1)
    with q = e^{0.8 s}, w = e^{0.8 d}, v2 = e^{0.2 d}.  The e^{0.2 s_i}
    row factor cancels in softmax.  The v2_j column factor is folded into
    the PE stationary [v2*hp | v2], so the per-element work is ONE
    tensor_scalar (4x DVE mode):  ea = max(q_i * w_j, 1)
    then ONE mask multiply (2x DVE):  da = ea * adjT
    and a PE matmul accumulating numerator and denominator together.

    ACT-offload variant for some (h,jb) blocks: ea_act = relu(q w - 1) on
    the Scalar engine; the missing "+1" is restored by an extra matmul of
    the SAME stationary [v2*hp | v2] against the raw adjT mask (exact).

adj mask trick: adj values 0.0/1.0 fp32; the high uint16 halves bitcast to
fp16 {0, 1.875} -- a constant scale on every surviving term that cancels
in the normalization.  Host passes adj TRANSPOSED as uint16 high halves.

Sharding: 8 cores = 4 batches x 2 head-pairs; each core handles 2 heads for
ALL 2048 query rows against all 2048 keys.  Output is fp16 (host upcasts);
the PSUM spill is scaled by 2^-6 which cancels in the num/den ratio.
"""

import os
from contextlib import ExitStack

import numpy as np

import concourse.bass as bass
import concourse.mybir as mybir
import concourse.tile as tile
from concourse import bacc
from concourse.bass_utils import run_bass_kernel_spmd
from concourse.masks import make_identity

F32 = mybir.dt.float32
F16 = mybir.dt.float16
U16 = mybir.dt.uint16
ALU = mybir.AluOpType
ACTF = mybir.ActivationFunctionType
AX = mybir.AxisListType

B, N, H, F = 4, 2048, 4, 64
NCORES = 8
ROWS = N               # query rows per core (full)
KEYS = N               # keys per core (full)
HEADS_PER = 2          # heads per core
NEG_SLOPE = 0.2
SPILL_SCALE = 2.0 ** -6


def default_assign(jb, h):
    """ea engine for block (jb, head h): "act" -> Scalar relu + PE fixup,
    "dve" -> Vector tensor_scalar max."""
    return "act" if (jb + h) % 2 == 0 else "dve"


def build_program(rows=ROWS, keys=KEYS, heads=HEADS_PER, f=F,
                  assign=default_assign):
    nc = bacc.Bacc("TRN2", target_bir_lowering=False, debug=False)

    kb = keys // 128           # key blocks
    qb = rows // 128           # query blocks (for q transposes)
    nhalf = rows // 512        # 512-wide output column chunks per head
    fe = f + 1                 # [v2*hp | v2] stationary width

    hbT_d = nc.dram_tensor("hbT", [f, keys], F32, kind="ExternalInput")
    adjt_d = nc.dram_tensor("adjt", [keys, rows], U16, kind="ExternalInput")
    w_d = nc.dram_tensor("wmat", [heads, f, f], F32, kind="ExternalInput")
    ap_d = nc.dram_tensor("apairt", [heads, 2, f], F32, kind="ExternalInput")
    out_d = nc.dram_tensor("out", [heads, rows, f], F16,
                           kind="ExternalOutput")

    with tile.TileContext(nc) as tc:
        with (
            tc.tile_pool(name="const", bufs=1) as const,
            tc.tile_pool(name="persist", bufs=1) as persist,
            tc.tile_pool(name="stmp", bufs=2) as stmp,
        ):
            id16 = const.tile([128, 128], F16, tag="id16")
            make_identity(nc, id16)
            neg1 = const.tile([128, 1], F32, tag="neg1")
            nc.vector.memset(neg1, -1.0)

            # ---- global loads -------------------------------------------
            hT32 = persist.tile([f, keys], F32, tag="hT32")
            nc.sync.dma_start(out=hT32, in_=hbT_d.ap())
            hT16 = persist.tile([f, keys], F16, tag="hT16")
            # cast in halves so the first hp matmul group starts earlier
            nc.scalar.activation(hT16[:, 0:keys // 2], hT32[:, 0:keys // 2],
                                 ACTF.Copy)
            nc.scalar.activation(hT16[:, keys // 2:], hT32[:, keys // 2:],
                                 ACTF.Copy)

            w32 = persist.tile([f, heads, f], F32, tag="w32")
            nc.sync.dma_start(out=w32, in_=w_d.ap().rearrange("h f o -> f h o"))
            w16 = persist.tile([f, heads, f], F16, tag="w16")
            nc.vector.tensor_copy(w16, w32)

            apr32 = persist.tile([1, heads, 2, f], F32, tag="apr32")
            nc.sync.dma_start(out=apr32, in_=ap_d.ap().unsqueeze(0))
            abc32 = persist.tile([128, heads, 2, f], F32, tag="abc32")
            nc.gpsimd.partition_broadcast(abc32, apr32)
            a16 = persist.tile([128, heads, 2, f], F16, tag="a16")
            nc.vector.tensor_copy(a16, abc32)

            # ---- prefetch all transposed mask blocks (streamed pool) ----
            # (declared early so DMA runs under the whole setup phase)
            adjp_stack = ExitStack()
            adjp = adjp_stack.enter_context(
                tc.tile_pool(name="adjp", bufs=8))
            adjts = []
            for jb in range(kb):
                adjt_t = adjp.tile([128, rows], U16, tag="adjt",
                                   name=f"adjt{jb}")
                nc.sync.dma_start(
                    out=adjt_t, in_=adjt_d.ap()[jb * 128:(jb + 1) * 128, :])
                adjts.append(adjt_t)

            # ---- per-head setup (head 0's chain completes first) --------
            tanh16 = persist.tile([128, heads, kb, f], F16, tag="tanh16")
            hpt2 = []    # [128, kb, fe] f16 -- [v2*hp | v2] stationaries
            wcol = []    # [128, kb] f32 -- e^{0.8 d}
            qbc = []     # [128, rows] f16 -- e^{0.8 s} broadcast
            with (
                tc.tile_pool(name="psum_hp", bufs=2, space="PSUM") as php,
                tc.tile_pool(name="psum_q", bufs=2, space="PSUM") as pq,
            ):
                for h in range(heads):
                    hpt_h = stmp.tile([128, kb, fe], F16, name=f"hptmp{h}",
                                      tag=f"hptmp{h}")
                    g_hp = min(8, kb)
                    for g in range(kb // g_hp):
                        pp = php.tile([128, g_hp * f], F32, tag="php")
                        for t in range(g_hp):
                            blk = g * g_hp + t
                            nc.tensor.matmul(
                                pp[:, t * f:(t + 1) * f],
                                lhsT=hT16[:, blk * 128:(blk + 1) * 128],
                                rhs=w16[:, h, :], start=True, stop=True)
                        nc.scalar.activation(
                            tanh16[:, h, g * g_hp:(g + 1) * g_hp, :],
                            pp.rearrange("p (t o) -> p t o", o=f),
                            ACTF.Tanh)
                        nc.scalar.activation(
                            hpt_h[:, g * g_hp:(g + 1) * g_hp, 0:f],
                            pp.rearrange("p (t o) -> p t o", o=f),
                            ACTF.Identity)
                    nc.vector.memset(hpt_h[:, :, f:fe], 1.0)

                    # s, d for this head: prod + blockwise reduce (s first:
                    # it unblocks the q chain before d finishes)
                    prod = stmp.tile([128, kb, 2, f], F16, name=f"prod{h}",
                                     tag=f"prod{h}")
                    nc.vector.tensor_tensor(
                        out=prod,
                        in0=tanh16[:, h].unsqueeze(2).broadcast_to(
                            [128, kb, 2, f]),
                        in1=a16[:, h].unsqueeze(1).broadcast_to(
                            [128, kb, 2, f]),
                        op=ALU.mult)
                    sums = stmp.tile([128, kb, 2], F32, name=f"sums{h}",
                                     tag=f"sums{h}")
                    nc.vector.reduce_sum(sums[:, :, 0:1], prod[:, :, 0:1, :],
                                         axis=AX.X)

                    # q chain: exp col (f16), transpose to row, evac,
                    # broadcast (gpsimd runs while DVE does the d reduce)
                    qc16 = stmp.tile([128, kb], F16, name=f"qc{h}",
                                     tag=f"qc{h}")
                    nc.scalar.activation(qc16, sums[:, :, 0], ACTF.Exp,
                                         scale=1.0 - NEG_SLOPE)
                    pq_t = pq.tile([1, rows], F16, tag="pq")
                    for t in range(qb):
                        nc.tensor.transpose(
                            pq_t[:, t * 128:(t + 1) * 128],
                            qc16[:, t:t + 1], id16)
                    qrow = stmp.tile([1, rows], F16, name=f"qrow{h}",
                                     tag=f"qrow{h}")
                    nc.vector.tensor_copy(qrow, pq_t)
                    qb_h = persist.tile([128, rows], F16, tag=f"qb{h}")
                    nc.gpsimd.partition_broadcast(qb_h, qrow)
                    qbc.append(qb_h)

                    # d reduce + w = e^{0.8 d} (scalar for ea), v2 = e^{0.2
                    # d} (f16, folded into the stationary)
                    nc.vector.reduce_sum(sums[:, :, 1:2], prod[:, :, 1:2, :],
                                         axis=AX.X)
                    w_h = persist.tile([128, kb], F32, tag=f"w{h}")
                    nc.scalar.activation(w_h, sums[:, :, 1], ACTF.Exp,
                                         scale=1.0 - NEG_SLOPE)
                    wcol.append(w_h)
                    v2c = stmp.tile([128, kb], F16, name=f"v2c{h}",
                                    tag=f"v2c{h}")
                    nc.scalar.activation(v2c, sums[:, :, 1], ACTF.Exp,
                                         scale=NEG_SLOPE)
                    hpt2_h = persist.tile([128, kb, fe], F16, tag=f"hpt2{h}")
                    nc.vector.tensor_tensor(
                        out=hpt2_h, in0=hpt_h,
                        in1=v2c.unsqueeze(2).broadcast_to([128, kb, fe]),
                        op=ALU.mult)
                    hpt2.append(hpt2_h)

            # ---- main loop: masked weights + fused matmul ---------------
            nacc = heads * nhalf
            acc_sb = persist.tile([fe, nacc, 512], F16, tag="acc_sb")
            accp_stack = ExitStack()
            accp = accp_stack.enter_context(
                tc.tile_pool(name="accp", bufs=1, space="PSUM"))
            accs = {}
            for h in range(heads):
                for half in range(nhalf):
                    i = h * nhalf + half
                    accs[i] = accp.tile([fe, 512], F32, tag=f"acc{i}",
                                        name=f"acc{i}")

            with (
                tc.tile_pool(name="ep", bufs=4) as ep,
                tc.tile_pool(name="dp", bufs=4) as dp,
            ):
                for jb in range(kb):
                    adj16 = adjts[jb].bitcast(F16)
                    ea = ep.tile([128, heads, rows], F16, tag="ea")
                    # ACT-path ea first so the Scalar engine starts early
                    order = sorted(range(heads),
                                   key=lambda h: assign(jb, h) != "act")
                    for h in order:
                        w_s = wcol[h][:, jb:jb + 1]
                        if assign(jb, h) == "act":
                            nc.scalar.activation(
                                ea[:, h, :], qbc[h], ACTF.Relu,
                                bias=neg1, scale=w_s)
                        else:
                            nc.vector.tensor_scalar(
                                out=ea[:, h, :], in0=qbc[h],
                                scalar1=w_s, scalar2=1.0,
                                op0=ALU.mult, op1=ALU.max)
                    da = dp.tile([128, heads, rows], F16, tag="da")
                    nc.vector.tensor_tensor(
                        out=da, in0=ea,
                        in1=adj16.unsqueeze(1).broadcast_to(
                            [128, heads, rows]),
                        op=ALU.mult)
                    last = jb == kb - 1
                    # act-extra matmuls first for jb>0: they depend only on
                    # the mask + stationary, so they fill PE gaps while da
                    # is still being computed
                    if jb > 0:
                        for h in range(heads):
                            if assign(jb, h) != "act":
                                continue
                            for half in range(nhalf):
                                sl = slice(half * 512, (half + 1) * 512)
                                nc.tensor.matmul(
                                    accs[h * nhalf + half],
                                    lhsT=hpt2[h][:, jb, :],
                                    rhs=adj16[:, sl],
                                    start=False, stop=False)
                    for h in range(heads):
                        is_act = assign(jb, h) == "act"
                        for half in range(nhalf):
                            sl = slice(half * 512, (half + 1) * 512)
                            nc.tensor.matmul(
                                accs[h * nhalf + half],
                                lhsT=hpt2[h][:, jb, :],
                                rhs=da[:, h, sl],
                                start=(jb == 0),
                                stop=(last and not (is_act and jb == 0)))
                            if is_act and jb == 0:
                                nc.tensor.matmul(
                                    accs[h * nhalf + half],
                                    lhsT=hpt2[h][:, jb, :],
                                    rhs=adj16[:, sl],
                                    start=False, stop=last)

                # spill accumulators (scaled; scale cancels in num/den)
                for i in range(nacc):
                    nc.scalar.activation(acc_sb[:, i, :], accs[i],
                                         ACTF.Identity, scale=SPILL_SCALE)
            accp_stack.close()
            adjp_stack.close()

            # ---- normalize + store (fp16) -------------------------------
            nq = 512 // 128
            with (
                tc.tile_pool(name="ptf", bufs=2, space="PSUM") as ptf,
                tc.tile_pool(name="outp", bufs=4) as outp,
            ):
                for h in range(heads):
                    for half in range(nhalf):
                        i = h * nhalf + half
                        pt = ptf.tile([128, nq, fe + 1], F16, tag="pt")
                        for q in range(nq):
                            nc.tensor.transpose(
                                pt[:, q, 0:fe],
                                acc_sb[:, i, q * 128:(q + 1) * 128],
                                id16[0:fe, 0:fe])
                        rcol = outp.tile([128, nq], F32, tag="rcol")
                        nc.vector.reciprocal(rcol, pt[:, :, f])
                        rc16 = outp.tile([128, nq], F16, tag="rc16")
                        nc.vector.tensor_copy(rc16, rcol)
                        osb = outp.tile([128, nq, f], F16, tag="osb")
                        nc.vector.tensor_tensor(
                            out=osb, in0=pt[:, :, 0:f],
                            in1=rc16.unsqueeze(2).broadcast_to(
                                [128, nq, f]),
                            op=ALU.mult)
                        nc.sync.dma_start(
                            out=out_d.ap()[
                                h, half * 512:(half + 1) * 512, :]
                            .rearrange("(q p) f -> p q f", p=128),
                            in_=osb)
    nc.compile()
    return nc


_PROGRAM_CACHE = {}


def _get_program():
    key = "full"
    if key not in _PROGRAM_CACHE:
        _PROGRAM_CACHE[key] = build_program()
    return _PROGRAM_CACHE[key]


def make_in_maps(h, adj, w, a_src, a_dst):
    """Shard + marshal the full inputs into 8 per-core input maps."""
    h = np.ascontiguousarray(np.asarray(h, dtype=np.float32))
    adj = np.ascontiguousarray(np.asarray(adj, dtype=np.float32))
    w = np.ascontiguousarray(np.asarray(w, dtype=np.float32))
    apairt_all = np.concatenate(
        [np.asarray(a_src)[:, None, :, 0], np.asarray(a_dst)[:, None, :, 0]],
        axis=1).astype(np.float32)  # [H, 2, F]
    in_maps = []
    hbT = {}
    adjt = {}
    for b in range(B):
        hbT[b] = np.ascontiguousarray(h[b].T)  # [F, N]
        adjT = np.ascontiguousarray(adj[b].T)  # [N, N] keys x queries
        adjt[b] = np.ascontiguousarray(
            adjT.view(np.uint16).reshape(N, N, 2)[:, :, 1])
    for c in range(NCORES):
        b, p = c // 2, c % 2
        in_maps.append({
            "hbT": hbT[b],
            "adjt": adjt[b],
            "wmat": np.ascontiguousarray(w[2 * p:2 * p + 2]),
            "apairt": np.ascontiguousarray(apairt_all[2 * p:2 * p + 2]),
        })
    return in_maps


def assemble_output(results, bias):
    """Gather per-core [2, N, F] fp16 results into [B, H, N, F] fp32."""
    out = np.empty((B, H, N, F), dtype=np.float32)
    for c in range(NCORES):
        b, p = c // 2, c % 2
        out[b, 2 * p:2 * p + 2, :, :] = results[c]["out"].astype(np.float32)
    if bias is not None:
        out = out + np.asarray(bias, dtype=np.float32)[None, None, None, :]
    return out


def run(h, adj, w, a_src, a_dst, bias, trace=False, trace_kwargs=None):
    nc = _get_program()
    in_maps = make_in_maps(h, adj, w, a_src, a_dst)
    res = run_bass_kernel_spmd(nc, in_maps, core_ids=list(range(NCORES)),
                               trace=trace, **(trace_kwargs or {}))
    return assemble_output(res.results, bias), res


def kernel(h, adj, w, a_src, a_dst, bias):
    out, _ = run(h, adj, w, a_src, a_dst, bias,
                 trace=bool(int(os.environ.get("GAT_TRACE", "0"))))
    return out


# revision 14
# speedup vs baseline: 1.0674x; 1.0353x over previous
"""Batched multi-head graph attention (GAT) kernel for 8 Trainium2 NeuronCores.

Math (per batch b, head h):
    hp      = h[b] @ w[h]                          # [N, F]
    t       = tanh(hp)
    s       = t @ a_src[h];  d = t @ a_dst[h]      # [N]
    score   = leaky_relu(s_i + d_j, 0.2)
    e       = where(adj>0, exp(score), 0)
    out     = (e / e.sum(-1, keepdim)) @ hp + bias

On-device identity (v2-folded form):
    exp(leaky(z)) = e^{0.2 s_i} * v2_j * max(q_i w_j, 1)
    with q = e^{0.8 s}, w = e^{0.8 d}, v2 = e^{0.2 d}.  The e^{0.2 s_i}
    row factor cancels in softmax.  The v2_j column factor is folded into
    the PE stationary [v2*hp | v2], so the per-element work is ONE
    tensor_scalar (4x DVE mode):  ea = max(q_i * w_j, 1)
    then ONE mask multiply (2x DVE):  da = ea * adjT
    and a PE matmul accumulating numerator and denominator together.

    ACT-offload variant for some (h,jb) blocks: ea_act = relu(q w - 1) on
    the Scalar engine; the missing "+1" is restored by an extra matmul of
    the SAME stationary [v2*hp | v2] against the raw adjT mask (exact).

adj mask trick: adj values 0.0/1.0 fp32; the high uint16 halves bitcast to
fp16 {0, 1.875} -- a constant scale on every surviving term that cancels
in the normalization.  Host passes adj TRANSPOSED as uint16 high halves.

Sharding: 8 cores = 4 batches x 2 head-pairs; each core handles 2 heads for
ALL 2048 query rows against all 2048 keys.  Output is fp16 (host upcasts);
the PSUM spill is scaled by 2^-6 which cancels in the num/den ratio.
"""

import os
from contextlib import ExitStack

import numpy as np

import concourse.bass as bass
import concourse.mybir as mybir
import concourse.tile as tile
from concourse import bacc
from concourse.bass_utils import run_bass_kernel_spmd
from concourse.masks import make_identity

F32 = mybir.dt.float32
F16 = mybir.dt.float16
U16 = mybir.dt.uint16
ALU = mybir.AluOpType
ACTF = mybir.ActivationFunctionType
AX = mybir.AxisListType

B, N, H, F = 4, 2048, 4, 64
NCORES = 8
ROWS = N               # query rows per core (full)
KEYS = N               # keys per core (full)
HEADS_PER = 2          # heads per core
NEG_SLOPE = 0.2
SPILL_SCALE = 2.0 ** -6


def default_assign(jb, h):
    """ea engine for block (jb, head h): "act" -> Scalar relu + PE fixup,
    "dve" -> Vector tensor_scalar max."""
    return "act" if (jb + h) % 2 == 0 else "dve"


def build_program(rows=ROWS, keys=KEYS, heads=HEADS_PER, f=F,
                  assign=default_assign):
    nc = bacc.Bacc("TRN2", target_bir_lowering=False, debug=False)

    kb = keys // 128           # key blocks
    qb = rows // 128           # query blocks (for q transposes)
    nhalf = rows // 512        # 512-wide output column chunks per head
    fe = f + 1                 # [v2*hp | v2] stationary width

    hbT_d = nc.dram_tensor("hbT", [f, keys], F32, kind="ExternalInput")
    adjt_d = nc.dram_tensor("adjt", [keys, rows], U16, kind="ExternalInput")
    w_d = nc.dram_tensor("wmat", [heads, f, f], F32, kind="ExternalInput")
    # host pre-broadcasts the attention vectors to all 128 partitions
    ap_d = nc.dram_tensor("apairb", [128, heads, 2, f], F32,
                          kind="ExternalInput")
    out_d = nc.dram_tensor("out", [heads, rows, f], F16,
                           kind="ExternalOutput")

    with tile.TileContext(nc) as tc:
        with (
            tc.tile_pool(name="const", bufs=1) as const,
            tc.tile_pool(name="persist", bufs=1) as persist,
            tc.tile_pool(name="stmp", bufs=2) as stmp,
        ):
            id16 = const.tile([128, 128], F16, tag="id16")
            make_identity(nc, id16)
            neg1 = const.tile([128, 1], F32, tag="neg1")
            nc.vector.memset(neg1, -1.0)

            # ---- global loads (small unblockers first) ------------------
            abc32 = persist.tile([128, heads, 2, f], F32, tag="abc32")
            nc.sync.dma_start(out=abc32, in_=ap_d.ap())
            hT32 = persist.tile([f, keys], F32, tag="hT32")
            nc.sync.dma_start(out=hT32, in_=hbT_d.ap())
            w32 = persist.tile([f, heads, f], F32, tag="w32")
            nc.sync.dma_start(out=w32, in_=w_d.ap().rearrange("h f o -> f h o"))

            a16 = persist.tile([128, heads, 2, f], F16, tag="a16")
            nc.vector.tensor_copy(a16, abc32)
            w16 = persist.tile([f, heads, f], F16, tag="w16")
            nc.vector.tensor_copy(w16, w32)
            hT16 = persist.tile([f, keys], F16, tag="hT16")
            # cast in halves so the first hp matmul group starts earlier
            nc.scalar.activation(hT16[:, 0:keys // 2], hT32[:, 0:keys // 2],
                                 ACTF.Copy)
            nc.scalar.activation(hT16[:, keys // 2:], hT32[:, keys // 2:],
                                 ACTF.Copy)

            # ---- prefetch all transposed mask blocks (streamed pool) ----
            # (declared early so DMA runs under the whole setup phase)
            adjp_stack = ExitStack()
            adjp = adjp_stack.enter_context(
                tc.tile_pool(name="adjp", bufs=8))
            adjts = []
            for jb in range(kb):
                adjt_t = adjp.tile([128, rows], U16, tag="adjt",
                                   name=f"adjt{jb}")
                nc.sync.dma_start(
                    out=adjt_t, in_=adjt_d.ap()[jb * 128:(jb + 1) * 128, :])
                adjts.append(adjt_t)

            # ---- per-head setup (head 0's chain completes first) --------
            tanh16 = persist.tile([128, heads, kb, f], F16, tag="tanh16")
            hpt2 = []    # [128, kb, fe] f16 -- [v2*hp | v2] stationaries
            wcol = []    # [128, kb] f32 -- e^{0.8 d}
            qbc = []     # [128, rows] f16 -- e^{0.8 s} broadcast
            with (
                tc.tile_pool(name="psum_hp", bufs=2, space="PSUM") as php,
                tc.tile_pool(name="psum_q", bufs=2, space="PSUM") as pq,
            ):
                for h in range(heads):
                    hpt_h = stmp.tile([128, kb, fe], F16, name=f"hptmp{h}",
                                      tag=f"hptmp{h}")
                    g_hp = min(8, kb)
                    for g in range(kb // g_hp):
                        pp = php.tile([128, g_hp * f], F32, tag="php")
                        for t in range(g_hp):
                            blk = g * g_hp + t
                            nc.tensor.matmul(
                                pp[:, t * f:(t + 1) * f],
                                lhsT=hT16[:, blk * 128:(blk + 1) * 128],
                                rhs=w16[:, h, :], start=True, stop=True)
                        nc.scalar.activation(
                            tanh16[:, h, g * g_hp:(g + 1) * g_hp, :],
                            pp.rearrange("p (t o) -> p t o", o=f),
                            ACTF.Tanh)
                        nc.scalar.activation(
                            hpt_h[:, g * g_hp:(g + 1) * g_hp, 0:f],
                            pp.rearrange("p (t o) -> p t o", o=f),
                            ACTF.Identity)
                    nc.vector.memset(hpt_h[:, :, f:fe], 1.0)

                    # s, d for this head: prod + blockwise reduce (s first:
                    # it unblocks the q chain before d finishes)
                    prod = stmp.tile([128, kb, 2, f], F16, name=f"prod{h}",
                                     tag=f"prod{h}")
                    nc.vector.tensor_tensor(
                        out=prod,
                        in0=tanh16[:, h].unsqueeze(2).broadcast_to(
                            [128, kb, 2, f]),
                        in1=a16[:, h].unsqueeze(1).broadcast_to(
                            [128, kb, 2, f]),
                        op=ALU.mult)
                    sums = stmp.tile([128, kb, 2], F32, name=f"sums{h}",
                                     tag=f"sums{h}")
                    nc.vector.reduce_sum(sums[:, :, 0:1], prod[:, :, 0:1, :],
                                         axis=AX.X)

                    # q chain: exp col (f16), transpose to row, evac,
                    # broadcast (gpsimd runs while DVE does the d reduce)
                    qc16 = stmp.tile([128, kb], F16, name=f"qc{h}",
                                     tag=f"qc{h}")
                    nc.scalar.activation(qc16, sums[:, :, 0], ACTF.Exp,
                                         scale=1.0 - NEG_SLOPE)
                    pq_t = pq.tile([1, rows], F16, tag="pq")
                    for t in range(qb):
                        nc.tensor.transpose(
                            pq_t[:, t * 128:(t + 1) * 128],
                            qc16[:, t:t + 1], id16)
                    qrow = stmp.tile([1, rows], F16, name=f"qrow{h}",
                                     tag=f"qrow{h}")
                    nc.vector.tensor_copy(qrow, pq_t)
                    qb_h = persist.tile([128, rows], F16, tag=f"qb{h}")
                    nc.gpsimd.partition_broadcast(qb_h, qrow)
                    qbc.append(qb_h)

                    # d reduce + w = e^{0.8 d} (scalar for ea), v2 = e^{0.2
                    # d} (f16, folded into the stationary)
                    nc.vector.reduce_sum(sums[:, :, 1:2], prod[:, :, 1:2, :],
                                         axis=AX.X)
                    w_h = persist.tile([128, kb], F32, tag=f"w{h}")
                    nc.scalar.activation(w_h, sums[:, :, 1], ACTF.Exp,
                                         scale=1.0 - NEG_SLOPE)
                    wcol.append(w_h)
                    v2c = stmp.tile([128, kb], F16, name=f"v2c{h}",
                                    tag=f"v2c{h}")
                    nc.scalar.activation(v2c, sums[:, :, 1], ACTF.Exp,
                                         scale=NEG_SLOPE)
                    hpt2_h = persist.tile([128, kb, fe], F16, tag=f"hpt2{h}")
                    nc.vector.tensor_tensor(
                        out=hpt2_h, in0=hpt_h,
                        in1=v2c.unsqueeze(2).broadcast_to([128, kb, fe]),
                        op=ALU.mult)
                    hpt2.append(hpt2_h)

            # ---- main loop: masked weights + fused matmul ---------------
            nacc = heads * nhalf
            acc_sb = persist.tile([fe, nacc, 512], F16, tag="acc_sb")
            accp_stack = ExitStack()
            accp = accp_stack.enter_context(
                tc.tile_pool(name="accp", bufs=1, space="PSUM"))
            accs = {}
            for h in range(heads):
                for half in range(nhalf):
                    i = h * nhalf + half
                    accs[i] = accp.tile([fe, 512], F32, tag=f"acc{i}",
                                        name=f"acc{i}")

            with (
                tc.tile_pool(name="ep", bufs=4) as ep,
                tc.tile_pool(name="dp", bufs=4) as dp,
            ):
                for jb in range(kb):
                    adj16 = adjts[jb].bitcast(F16)
                    last = jb == kb - 1
                    for h in range(heads):
                        is_act = assign(jb, h) == "act"
                        w_s = wcol[h][:, jb:jb + 1]
                        ea = ep.tile([128, rows], F16, tag=f"ea{h}",
                                     name=f"ea{h}_{jb}")
                        if is_act:
                            nc.scalar.activation(
                                ea, qbc[h], ACTF.Relu,
                                bias=neg1, scale=w_s)
                        else:
                            nc.vector.tensor_scalar(
                                out=ea, in0=qbc[h],
                                scalar1=w_s, scalar2=1.0,
                                op0=ALU.mult, op1=ALU.max)
                        da = dp.tile([128, rows], F16, tag=f"da{h}",
                                     name=f"da{h}_{jb}")
                        nc.vector.tensor_tensor(out=da, in0=ea, in1=adj16,
                                                op=ALU.mult)
                        for half in range(nhalf):
                            sl = slice(half * 512, (half + 1) * 512)
                            nc.tensor.matmul(
                                accs[h * nhalf + half],
                                lhsT=hpt2[h][:, jb, :],
                                rhs=da[:, sl],
                                start=(jb == 0),
                                stop=(last and not is_act))
                            if is_act:
                                nc.tensor.matmul(
                                    accs[h * nhalf + half],
                                    lhsT=hpt2[h][:, jb, :],
                                    rhs=adj16[:, sl],
                                    start=False, stop=last)

                # spill accumulators (scaled; scale cancels in num/den)
                for i in range(nacc):
                    nc.scalar.activation(acc_sb[:, i, :], accs[i],
                                         ACTF.Identity, scale=SPILL_SCALE)
            accp_stack.close()
            adjp_stack.close()

            # ---- normalize + store (fp16) -------------------------------
            nq = 512 // 128
            with (
                tc.tile_pool(name="ptf", bufs=2, space="PSUM") as ptf,
                tc.tile_pool(name="outp", bufs=4) as outp,
            ):
                for h in range(heads):
                    for half in range(nhalf):
                        i = h * nhalf + half
                        pt = ptf.tile([128, nq, fe + 1], F16, tag="pt")
                        for q in range(nq):
                            nc.tensor.transpose(
                                pt[:, q, 0:fe],
                                acc_sb[:, i, q * 128:(q + 1) * 128],
                                id16[0:fe, 0:fe])
                        rcol = outp.tile([128, nq], F32, tag="rcol")
                        nc.vector.reciprocal(rcol, pt[:, :, f])
                        rc16 = outp.tile([128, nq], F16, tag="rc16")
                        nc.vector.tensor_copy(rc16, rcol)
                        osb = outp.tile([128, nq, f], F16, tag="osb")
                        nc.vector.tensor_tensor(
                            out=osb, in0=pt[:, :, 0:f],
                            in1=rc16.unsqueeze(2).broadcast_to(
                                [128, nq, f]),
                            op=ALU.mult)
                        nc.sync.dma_start(
                            out=out_d.ap()[
                                h, half * 512:(half + 1) * 512, :]
                            .rearrange("(q p) f -> p q f", p=128),
                            in_=osb)
    nc.compile()
    return nc


_PROGRAM_CACHE = {}


def _get_program():
    key = "full"
    if key not in _PROGRAM_CACHE:
        _PROGRAM_CACHE[key] = build_program()
    return _PROGRAM_CACHE[key]


def make_in_maps(h, adj, w, a_src, a_dst):
    """Shard + marshal the full inputs into 8 per-core input maps."""
    h = np.ascontiguousarray(np.asarray(h, dtype=np.float32))
    adj = np.ascontiguousarray(np.asarray(adj, dtype=np.float32))
    w = np.ascontiguousarray(np.asarray(w, dtype=np.float32))
    apairt_all = np.concatenate(
        [np.asarray(a_src)[:, None, :, 0], np.asarray(a_dst)[:, None, :, 0]],
        axis=1).astype(np.float32)  # [H, 2, F]
    in_maps = []
    hbT = {}
    adjt = {}
    for b in range(B):
        hbT[b] = np.ascontiguousarray(h[b].T)  # [F, N]
        adjT = np.ascontiguousarray(adj[b].T)  # [N, N] keys x queries
        adjt[b] = np.ascontiguousarray(
            adjT.view(np.uint16).reshape(N, N, 2)[:, :, 1])
    for c in range(NCORES):
        b, p = c // 2, c % 2
        apair = apairt_all[2 * p:2 * p + 2]  # [2, 2, F]
        apairb = np.ascontiguousarray(
            np.broadcast_to(apair[None], (128,) + apair.shape))
        in_maps.append({
            "hbT": hbT[b],
            "adjt": adjt[b],
            "wmat": np.ascontiguousarray(w[2 * p:2 * p + 2]),
            "apairb": apairb,
        })
    return in_maps


def assemble_output(results, bias):
    """Gather per-core [2, N, F] fp16 results into [B, H, N, F] fp32."""
    out = np.empty((B, H, N, F), dtype=np.float32)
    for c in range(NCORES):
        b, p = c // 2, c % 2
        out[b, 2 * p:2 * p + 2, :, :] = results[c]["out"].astype(np.float32)
    if bias is not None:
        out = out + np.asarray(bias, dtype=np.float32)[None, None, None, :]
    return out


def run(h, adj, w, a_src, a_dst, bias, trace=False, trace_kwargs=None):
    nc = _get_program()
    in_maps = make_in_maps(h, adj, w, a_src, a_dst)
    res = run_bass_kernel_spmd(nc, in_maps, core_ids=list(range(NCORES)),
                               trace=trace, **(trace_kwargs or {}))
    return assemble_output(res.results, bias), res


def kernel(h, adj, w, a_src, a_dst, bias):
    out, _ = run(h, adj, w, a_src, a_dst, bias,
                 trace=bool(int(os.environ.get("GAT_TRACE", "0"))))
    return out
